# revision 3
# baseline (speedup 1.0000x reference)
"""GCN layer (gather + segment-sum + Linear) as a Bass kernel on 8 trn2 cores.

out[v] = (sum_{e: dst[e]==v} node_feats[src[e]]) @ W.T + b

Strategy (dst-sharded, no collective needed):
  - Core c owns dst nodes [c*6250, (c+1)*6250). Its ~100K edges are grouped by
    128-node dst windows (49 per core) on the host (pure integer prep).
  - Node features are staged as a bf16 table in HBM (row 0 / row 50001 are zero
    rows used for padding). dma_gather (SWDGE) fetches each edge's src row into
    SBUF with edges on partitions: msg[k%128, k//128, :]. int16 gather indices
    only reach 32767 rows, so edges are split into a "lo" (src <= 32766) and
    "hi" (src >= 32767) stream gathered from two base offsets of the table.
  - Per 128-edge chunk, a one-hot matrix S[k, d] = (dst_local[k] == d) is built
    on DVE with a single broadcast is_equal per window, and PE accumulates
    agg[feat, d] += msg_chunk.T @ S_chunk into PSUM over all chunks of the
    window (exactly segment-sum).
  - The Linear layer is fused per window: agg (cast bf16) is the lhsT of a
    second matmul with W.T, bias is added by DVE, result DMA'd to the output.
"""

import numpy as np
import ml_dtypes

N_NODES = 50000
N_EDGES = 800000
F = 128
NCORES = 8
ND = N_NODES // NCORES          # dst nodes per core
NW = (ND + 127) // 128          # dst windows per core
LO_MAX = 32766                  # src <= LO_MAX handled by lo gather (idx = src+1)
HI_BASE = 32768                 # hi gather base row; idx = src - 32767
HI_PAD = 17233                  # hi pad idx -> table row 50001 (zero row)
TABLE_ROWS = N_NODES + 2
GROUP = 8                       # dst windows per dma_gather call

BF16 = ml_dtypes.bfloat16

_cache = {}


def _host_prep(node_feats, src, dst):
    src = np.asarray(src, dtype=np.int64)
    dst = np.asarray(dst, dtype=np.int64)

    table = np.zeros((TABLE_ROWS, F), dtype=BF16)
    table[1 : N_NODES + 1] = np.asarray(node_feats, dtype=np.float32)

    core = dst // ND
    rel = dst - core * ND
    w = rel >> 7
    dl = rel & 127
    half = (src > LO_MAX).astype(np.int64)

    key = (core * NW + w) * 2 + half
    order = np.argsort(key, kind="stable")
    ks = key[order]
    srcs = src[order]
    dls = dl[order]

    cnt = np.bincount(ks, minlength=NCORES * NW * 2)
    lo_cnt = cnt[0::2]
    hi_cnt = cnt[1::2]
    L = int(np.ceil(lo_cnt.max() / 128))
    H = int(np.ceil(hi_cnt.max() / 128))
    C = L + H

    starts = np.zeros(NCORES * NW * 2, dtype=np.int64)
    np.cumsum(cnt[:-1], out=starts[1:])
    pos = np.arange(N_EDGES, dtype=np.int64) - starts[ks]

    idxs_lo = np.zeros((NCORES, NW, L * 128), dtype=np.int16)
    idxs_hi = np.full((NCORES, NW, H * 128), HI_PAD, dtype=np.int16)
    dstloc = np.zeros((NCORES, 128, NW * C), dtype=BF16)

    grp = ks >> 1
    cc = grp // NW
    ww = grp % NW
    lo_m = (ks & 1) == 0
    hi_m = ~lo_m

    idxs_lo[cc[lo_m], ww[lo_m], pos[lo_m]] = (srcs[lo_m] + 1).astype(np.int16)
    dstloc[cc[lo_m], pos[lo_m] & 127, ww[lo_m] * C + (pos[lo_m] >> 7)] = dls[lo_m]
    idxs_hi[cc[hi_m], ww[hi_m], pos[hi_m]] = (srcs[hi_m] - (LO_MAX + 1)).astype(np.int16)
    dstloc[cc[hi_m], pos[hi_m] & 127, ww[hi_m] * C + L + (pos[hi_m] >> 7)] = dls[hi_m]

    def wrap(flat):
        # gather idx layout: idx i -> [i % 16, i // 16], replicated x8 partitions
        a = flat.reshape(-1, 16).T
        return np.tile(a, (8, 1)).copy()

    idxs_lo_w = [wrap(idxs_lo[c].reshape(-1)) for c in range(NCORES)]
    idxs_hi_w = [wrap(idxs_hi[c].reshape(-1)) for c in range(NCORES)]
    return table, idxs_lo_w, idxs_hi_w, dstloc, L, H


def _build_program(L, H):
    import concourse.bass as bass
    import concourse.bacc as bacc
    import concourse.tile as tile
    import concourse.mybir as mybir
    from contextlib import ExitStack

    C = L + H
    dtB = mybir.dt.bfloat16
    dtF = mybir.dt.float32
    dtI = mybir.dt.int16
    Copy = mybir.ActivationFunctionType.Copy

    nc = bacc.Bacc("TRN2", target_bir_lowering=False, debug=False, num_devices=NCORES)

    table_d = nc.dram_tensor("table", [TABLE_ROWS, F], dtB, kind="ExternalInput")
    idxlo_d = nc.dram_tensor("idxs_lo", [128, NW * L * 8], dtI, kind="ExternalInput")
    idxhi_d = nc.dram_tensor("idxs_hi", [128, NW * H * 8], dtI, kind="ExternalInput")
    dst_d = nc.dram_tensor("dstloc", [128, NW * C], dtB, kind="ExternalInput")
    iota_d = nc.dram_tensor("iota", [128, C * 128], dtB, kind="ExternalInput")
    wt_d = nc.dram_tensor("wt", [F, F], dtB, kind="ExternalInput")
    b_d = nc.dram_tensor("bias", [128, F], dtF, kind="ExternalInput")
    out_d = nc.dram_tensor("out", [ND, F], dtF, kind="ExternalOutput")

    ngroups = (NW + GROUP - 1) // GROUP

    with tile.TileContext(nc) as tc:
        with ExitStack() as ctx:
            const = ctx.enter_context(tc.tile_pool(name="const", bufs=1))
            mlo_p = ctx.enter_context(tc.tile_pool(name="mlo", bufs=2))
            mhi_p = ctx.enter_context(tc.tile_pool(name="mhi", bufs=2))
            s_p = ctx.enter_context(tc.tile_pool(name="s", bufs=3))
            aggsb_p = ctx.enter_context(tc.tile_pool(name="aggsb", bufs=3))
            outsb_p = ctx.enter_context(tc.tile_pool(name="outsb", bufs=3))
            ps_agg = ctx.enter_context(tc.tile_pool(name="ps_agg", bufs=2, space="PSUM"))
            ps_out = ctx.enter_context(tc.tile_pool(name="ps_out", bufs=2, space="PSUM"))

            idxlo_t = const.tile([128, NW * L * 8], dtI)
            nc.sync.dma_start(idxlo_t[:], idxlo_d[:])
            idxhi_t = const.tile([128, NW * H * 8], dtI)
            nc.sync.dma_start(idxhi_t[:], idxhi_d[:])
            dst_t = const.tile([128, NW * C], dtB)
            nc.sync.dma_start(dst_t[:], dst_d[:])
            iota_t = const.tile([128, C * 128], dtB)
            nc.sync.dma_start(iota_t[:], iota_d[:])
            wt_t = const.tile([F, F], dtB)
            nc.sync.dma_start(wt_t[:], wt_d[:])
            b_t = const.tile([128, F], dtF)
            nc.sync.dma_start(b_t[:], b_d[:])

            ia = iota_t[:]
            iota_3d = bass.AP(ia.tensor, ia.offset, [list(ia.ap[0]), [128, C], [1, 128]])

            for g in range(ngroups):
                w0 = g * GROUP
                w1 = min(w0 + GROUP, NW)
                gw = w1 - w0

                lo_t = mlo_p.tile([128, GROUP * L, F], dtB, tag="mlo")
                nc.gpsimd.dma_gather(
                    lo_t[:, : gw * L, :], table_d[:],
                    idxlo_t[:, w0 * L * 8 : w1 * L * 8],
                    gw * L * 128, gw * L * 128, F, single_packet=False,
                )
                hi_t = mhi_p.tile([128, GROUP * H, F], dtB, tag="mhi")
                nc.gpsimd.dma_gather(
                    hi_t[:, : gw * H, :], table_d[HI_BASE:TABLE_ROWS, :],
                    idxhi_t[:, w0 * H * 8 : w1 * H * 8],
                    gw * H * 128, gw * H * 128, F, single_packet=False,
                )

                for wi in range(gw):
                    w = w0 + wi
                    s_t = s_p.tile([128, C, 128], dtB)
                    da = dst_t[:, w * C : (w + 1) * C]
                    dst_bc = bass.AP(da.tensor, da.offset,
                                     [list(da.ap[0]), list(da.ap[1]), [0, 128]])
                    nc.vector.tensor_tensor(s_t[:], iota_3d, dst_bc,
                                            mybir.AluOpType.is_equal)

                    agg = ps_agg.tile([F, 128], dtF)
                    for j in range(L):
                        nc.tensor.matmul(agg[:], lo_t[:, wi * L + j, :], s_t[:, j, :],
                                         start=(j == 0), stop=False)
                    for j in range(H):
                        nc.tensor.matmul(agg[:], hi_t[:, wi * H + j, :], s_t[:, L + j, :],
                                         start=False, stop=(j == H - 1))

                    aggsb = aggsb_p.tile([F, 128], dtB)
                    nc.scalar.activation(aggsb[:], agg[:], Copy)

                    out2 = ps_out.tile([128, F], dtF)
                    nc.tensor.matmul(out2[:], aggsb[:], wt_t[:], start=True, stop=True)

                    outsb = outsb_p.tile([128, F], dtF)
                    nc.vector.tensor_tensor(outsb[:], out2[:], b_t[:],
                                            mybir.AluOpType.add)

                    rows = min(128, ND - w * 128)
                    nc.sync.dma_start(out_d[w * 128 : w * 128 + rows, :],
                                      outsb[:rows, :])

    nc.compile()
    return nc


def _run(node_feats, src, dst, W, b, trace=False, trace_kwargs=None):
    from concourse.bass_utils import run_bass_kernel_spmd

    table, idxs_lo_w, idxs_hi_w, dstloc, L, H = _host_prep(node_feats, src, dst)
    C = L + H

    key = (L, H)
    if key not in _cache:
        _cache[key] = _build_program(L, H)
    nc = _cache[key]

    iota = np.tile(np.arange(128, dtype=np.float32), (128, C)).astype(BF16)
    wt = np.asarray(W, dtype=np.float32).T.astype(BF16).copy()
    b_bc = np.tile(np.asarray(b, dtype=np.float32), (128, 1)).copy()

    in_maps = [
        {
            "table": table,
            "idxs_lo": idxs_lo_w[c],
            "idxs_hi": idxs_hi_w[c],
            "dstloc": np.ascontiguousarray(dstloc[c]),
            "iota": iota,
            "wt": wt,
            "bias": b_bc,
        }
        for c in range(NCORES)
    ]

    res = run_bass_kernel_spmd(nc, in_maps, list(range(NCORES)), trace=trace,
                               **(trace_kwargs or {}))
    out = np.concatenate([res.results[c]["out"] for c in range(NCORES)], axis=0)
    return out, res


def kernel(node_feats, edge_feats, src, dst, W, b):
    out, _ = _run(node_feats, src, dst, W, b, trace=False)
    return out.astype(np.float32)
